# revision 78
# baseline (speedup 1.0000x reference)
"""Cross-attention (cosine/l2-normalized, biased softmax) on 8 TRN2 NeuronCores.

Sharding: core c handles batch b = c//2 and head group g = c%2 (8 of 16 heads,
i.e. a 512-wide slice of the QKV projections / Wo rows).  Each core computes a
partial output (its heads' contribution through Wo); the host sums the two
partials per batch and adds bo.

All tensors are kept transposed on chip (channels on partitions):
  qT/kT = (Wx)^T computed as lhsT=W, rhs=x^T; v in natural [j, ch] layout via
  lhsT=y^T.  Scores are computed transposed S^T[j, i] (lhsT = kn^T slice,
  rhs = qn^T slice, K = 64), softmax runs unnormalized as exp(S)*exp(bias)
  with the denominator obtained from an extra all-ones column appended to V,
  and the division happens after the PV matmul (partition_broadcast + mul).
L2-norm denominators use a block-diagonal selector matmul (K=128) for the
per-head sum of squares and exp(-0.5*ln(x)) on the scalar engine (the DVE
reciprocal op is ~5x an ACT pass; ACT Rsqrt is disallowed).

With KERNEL_ATTN_DT=bf16 (default) the inputs/weights are host-cast to bf16
so every matmul runs at the PE's full bf16 rate and input DMA traffic is
halved; PSUM accumulation, l2-norm chains and softmax denominators stay f32.

Scheduling notes (engines execute in queue order with shallow OOO windows,
and the PE clock drops to 1.2 GHz after any idle gap, reaching 2.4 GHz only
after 3us of continuous work — so the emission order is arranged to keep
every queue's head ready):
  - DMA is issued per-k-tile, first-needed first; eb/wo prefetch during the
    projection phases.
  - Each l2-norm chain (DVE square -> selector matmul -> ACT ln/exp -> mul)
    is split and emitted behind the next tile's raw matmuls.
  - The attention phase is ACT(exp)-bound: per-sweep softmax normalization
    is p-batched and deferred into the next sweep (bct/oT-mul emitted at
    jt=3/10) so it never gates the exp stream; gpsimd runs ONLY
    partition_broadcasts there (a dummy broadcast prewarms the Q7 ucode --
    switching gpsimd op types mid-phase costs ~7us per switch).
  - ic1's first sweep is emitted before ic0's output projection so the PE
    never waits on the trailing normalize chain; psf has its own psum banks
    so the final out-projection starts barrier-free.
"""

import os
import numpy as np
import ml_dtypes

import concourse.bass as bass
import concourse.tile as tile
from concourse import bacc, mybir
from concourse.bass_utils import run_bass_kernel_spmd


def _ensure_ntff_hook():
    """Some containers ship an `antenv` stub without `axon_hooks`; trn_boot
    then skips installing the NTFF profile hook and run_bass_kernel_spmd
    crashes on `trace=True`.  Recreate the module + hook (trn_boot step 6)
    when — and only when — it is missing."""
    try:
        import antenv.axon_hooks  # noqa: F401
        return
    except ImportError:
        pass
    try:
        import sys
        import types
        import antenv
        from trn_agent_boot.trn_boot import _ntff_profile_via_ctypes
        mod = types.ModuleType("antenv.axon_hooks")
        _state = {"hook": None}
        mod.set_axon_ntff_profile_hook = lambda h: _state.__setitem__("hook", h)
        mod.get_axon_ntff_profile_hook = lambda: _state["hook"]
        sys.modules["antenv.axon_hooks"] = mod
        antenv.axon_hooks = mod
        hook = _ntff_profile_via_ctypes("/opt/axon/libaxon_pjrt.so")
        if hook is not None:
            mod.set_axon_ntff_profile_hook(hook)
    except Exception:
        pass

F32 = mybir.dt.float32
F32R = mybir.dt.float32r
BF16 = mybir.dt.bfloat16

B, Lq, Ly, C = 4, 1024, 2048, 1024
H_TOT, D = 16, 64
HL = 8           # heads per core
CHL = HL * D     # 512 channels per core
TCH = CHL // 128  # 4 channel tiles (2 heads each)
KT = C // 128     # 8 contraction tiles for the projections
NJT = Ly // 128   # 16 j tiles
NIC = Lq // 512   # 2 i chunks
N_CORES = 8
MAX_SCALE_MUL = float(np.log(100.0))

# attention dtype: "f32" (f32r matmuls, fp32 probs) or "bf16"
ATTN_DT = os.environ.get("KERNEL_ATTN_DT", "bf16")

LAST_EXEC_NS = None
LAST_RES = None
_COMPILED = {}
Exp = mybir.ActivationFunctionType.Exp
Ln = mybir.ActivationFunctionType.Ln

_ACT_TABLES_INSTALLED = False


def _install_act_tables():
    """Point both bacc and walrus at an act_info.json with the combined
    ln+exp function set first, so Ln/Exp alternation (softmax denominators,
    l2-norm rsqrt via exp(-0.5 ln x)) stops thrashing the ACT spline table
    (~1.3 us per reload).  Selection is first-match over the set list."""
    global _ACT_TABLES_INSTALLED
    if _ACT_TABLES_INSTALLED:
        return
    import json
    import shutil
    import tempfile
    import concourse.hw_specs as hw_specs
    import concourse.bacc as bacc_mod
    try:
        from neuronxcc.driver.Job import Job
        from neuronxcc.driver.jobs.support.FindActInfo import findActInfoFile
        src = findActInfoFile(Job.getPackageDir(), "gen3")
    except Exception:
        return
    dst = os.path.join(tempfile.mkdtemp(prefix="actpwp"), "pwp")
    shutil.copytree(os.path.dirname(src), dst)
    info_path = os.path.join(dst, "act_info.json")
    with open(info_path) as f:
        info = json.load(f)
    key = "natural_log_exp_and_others"
    info["act_func_sets"].sort(key=lambda s: 0 if s["name"] == key else 1)
    with open(info_path, "w") as f:
        json.dump(info, f)
    os.environ["BASS_ACT_ROOT_JSON_PATH"] = info_path

    orig = hw_specs.get_activation_tables

    def reordered(arch):
        d = orig(arch)
        if key not in d:
            return d
        out = {key: d[key]}
        out.update((k, v) for k, v in d.items() if k != key)
        return out

    hw_specs.get_activation_tables = reordered
    bacc_mod.get_activation_tables = reordered
    _ACT_TABLES_INSTALLED = True


def _build(attn_dt: str):
    _install_act_tables()
    bf = attn_dt != "f32"
    AD = BF16 if bf else F32

    def mmcast(ap):
        # matmul operand dtype for the attention matmuls
        return ap if bf else ap.bitcast(F32R)

    nc = bacc.Bacc("TRN2", target_bir_lowering=False, debug=False,
                   num_devices=N_CORES)

    xT_ap = nc.dram_tensor("xT", [C, Lq], AD, kind="ExternalInput").ap()
    yT_ap = nc.dram_tensor("yT", [C, Ly], AD, kind="ExternalInput").ap()
    wq_ap = nc.dram_tensor("wq", [C, CHL], AD, kind="ExternalInput").ap()
    wk_ap = nc.dram_tensor("wk", [C, CHL], AD, kind="ExternalInput").ap()
    wv_ap = nc.dram_tensor("wv", [C, CHL], AD, kind="ExternalInput").ap()
    wo_ap = nc.dram_tensor("wo", [CHL, C], AD, kind="ExternalInput").ap()
    bq_ap = nc.dram_tensor("bq4", [128, TCH], F32, kind="ExternalInput").ap()
    is2_ap = nc.dram_tensor("invs2", [128, TCH], F32, kind="ExternalInput").ap()
    eb_ap = nc.dram_tensor("ebT", [Ly, Lq], AD, kind="ExternalInput").ap()
    out_ap = nc.dram_tensor("outT", [C, Lq], F32, kind="ExternalOutput").ap()

    xT_d = xT_ap.rearrange("(k p) i -> p k i", p=128)
    yT_d = yT_ap.rearrange("(k p) j -> p k j", p=128)
    wq_d = wq_ap.rearrange("(k p) n -> p k n", p=128)
    wk_d = wk_ap.rearrange("(k p) n -> p k n", p=128)
    wv_d = wv_ap.rearrange("(k p) n -> p k n", p=128)
    wo_d = wo_ap.rearrange("(k p) n -> p k n", p=128)
    eb_d = eb_ap.rearrange("(jt p) i -> p jt i", p=128)
    out_d = out_ap.rearrange("(ct p) i -> p ct i", p=128)

    with tile.TileContext(nc) as tc:
        with tc.tile_pool(name="persist", bufs=1) as pers, \
             tc.tile_pool(name="qn_p", bufs=1) as qn_p, \
             tc.tile_pool(name="kn_p", bufs=1) as kn_p, \
             tc.tile_pool(name="v_p", bufs=1) as v_p, \
             tc.tile_pool(name="wo_p", bufs=1) as wo_p, \
             tc.tile_pool(name="oT_p", bufs=1) as oT_p, \
             tc.tile_pool(name="eb_p", bufs=17) as eb_p:

            # block-diagonal parity selector: sel.T @ sq sums each 64-row
            # head block and replicates the sums over that block's rows
            sel_f = pers.tile([128, 128], F32)
            nc.gpsimd.memset(sel_f[:], 0.0)
            nc.gpsimd.memset(sel_f[0:64, 0:64], 1.0)
            nc.gpsimd.memset(sel_f[64:128, 64:128], 1.0)
            sel_r = pers.tile([128, 128], F32)
            nc.vector.tensor_copy(sel_r[:].bitcast(F32R), sel_f[:])
            bq_sb = pers.tile([128, TCH], F32)
            nc.sync.dma_start(bq_sb[:], bq_ap[:])
            is2_sb = pers.tile([128, TCH], F32)
            nc.sync.dma_start(is2_sb[:], is2_ap[:])

            qn_sb = qn_p.tile([128, TCH, Lq], AD)     # qn^T
            kn_sb = kn_p.tile([128, TCH, Ly], AD)     # kn^T
            v_sb = v_p.tile([128, NJT, HL * 65], AD)  # v (+ ones col per head)
            # the per-head ones columns never change: write them once
            nc.gpsimd.memset(
                v_sb[:].rearrange("p j (h e) -> p j h e", e=65)[:, :, :, 64:65],
                1.0)

            # attention-phase tensors are allocated up front so the eb / wo
            # prefetch can overlap the projection phases
            wo_sb = wo_p.tile([128, TCH, C], AD)
            oT_sb = oT_p.tile([128, TCH, Lq], AD)

            def emit_eb(ic):
                ebs = []
                for jt in range(NJT):
                    if bf:
                        # duplicated halves so the prob multiply is a
                        # plain step-1 2D op (DVE 2x bf16 mode)
                        ebt = eb_p.tile([128, 1024], AD, tag="eb",
                                        name=f"eb{ic}_{jt}")
                        nc.sync.dma_start(
                            ebt[:, 0:512],
                            eb_d[:, jt, ic * 512:(ic + 1) * 512])
                        nc.sync.dma_start(
                            ebt[:, 512:1024],
                            eb_d[:, jt, ic * 512:(ic + 1) * 512])
                    else:
                        ebt = eb_p.tile([128, 512], AD, tag="eb",
                                        name=f"eb{ic}_{jt}")
                        nc.sync.dma_start(
                            ebt[:], eb_d[:, jt, ic * 512:(ic + 1) * 512])
                    ebs.append(ebt)
                return ebs

            # yT/wv/wk space is reserved up front; their DMAs are emitted
            # after the q-phase loads so x/wq win the DMA queues first.
            with tc.tile_pool(name="yT_p", bufs=1) as yT_p, \
                 tc.tile_pool(name="wv_p", bufs=1) as wv_p, \
                 tc.tile_pool(name="wk_p", bufs=1) as wk_p:
                yT_sb = yT_p.tile([128, KT, Ly], AD)
                wv_sb = wv_p.tile([128, KT, CHL], AD)
                wk_sb = wk_p.tile([128, KT, CHL], AD)

                # -------------- Q projection + l2norm(+scale) --------------
                # qtmp/qnrm/psqs outlive the Q block: q tile 3's norm chain
                # is emitted inside the K phase (PSUM: Q 4+2, KV 2+2+2+2).
                # kraw/psk are allocated below the Q-phase pools so the
                # first K matmuls never wait on a space-reuse barrier
                # against xT/wq/psq (PSUM: Q 4+2+2 = 8, KV 2+2+2+2 = 8)
                with tc.tile_pool(name="qtmp", bufs=2) as qtmp, \
                     tc.tile_pool(name="qnrm", bufs=1) as qnrm, \
                     tc.tile_pool(name="kraw_p", bufs=2) as kraw_p, \
                     tc.tile_pool(name="psqs", bufs=2, space="PSUM") as psqs, \
                     tc.tile_pool(name="psk", bufs=2, space="PSUM") as psk:

                    def q_sq(t, qraw):
                        sq = qnrm.tile([128, Lq], F32, tag="sq",
                                       name=f"sq{t}")
                        nc.vector.tensor_mul(sq[:].bitcast(F32R), qraw[:],
                                             qraw[:])
                        return sq

                    def q_fin(t, qraw, sq):
                        rs = qnrm.tile([128, Lq], F32, tag="rs",
                                       name=f"rs{t}")
                        for ic in range(NIC):
                            ssq = psqs.tile([128, 512], F32)
                            nc.tensor.matmul(
                                ssq[:], sel_r[:].bitcast(F32R),
                                sq[:, ic * 512:(ic + 1) * 512].bitcast(F32R),
                                start=True, stop=True)
                            # s_h/|q| = exp(-0.5*ln(sumsq/s_h^2))
                            nc.scalar.activation(
                                rs[:, ic * 512:(ic + 1) * 512], ssq[:], Ln,
                                scale=is2_sb[:, t:t + 1])
                        nc.scalar.activation(rs[:], rs[:], Exp, scale=-0.5)
                        # Pool: keeps the in-order DVE queue free for the
                        # psum-draining adds/copies (Pool only runs
                        # tensor_tensor during the projection phases, so no
                        # gpsimd ucode swap cost)
                        nc.gpsimd.tensor_mul(mmcast(qn_sb[:, t, :]),
                                             qraw[:], rs[:])

                    with tc.tile_pool(name="xT_p", bufs=1) as xT_p, \
                         tc.tile_pool(name="wq_p", bufs=1) as wq_p, \
                         tc.tile_pool(name="psq", bufs=4, space="PSUM") as psq:
                        wq_sb = wq_p.tile([128, KT, CHL], AD)
                        xT_sb = xT_p.tile([128, KT, Lq], AD)
                        # per-k interleave so the first Q matmul's operands
                        # land as early as possible
                        for k in range(KT):
                            nc.sync.dma_start(wq_sb[:, k, :], wq_d[:, k, :])
                            nc.sync.dma_start(xT_sb[:, k, :], xT_d[:, k, :])
                        for k in range(KT):
                            nc.sync.dma_start(wk_sb[:, k, :], wk_d[:, k, :])
                            nc.sync.dma_start(yT_sb[:, k, :], yT_d[:, k, :])
                        nc.sync.dma_start(wv_sb[:], wv_d[:])
                        # prefetch ic=0 bias tiles and wo during projections
                        ebs0 = emit_eb(0)
                        nc.sync.dma_start(wo_sb[:], wo_d[:])

                        def q_raw(t):
                            qraw = qtmp.tile([128, Lq], F32, tag="qraw",
                                             name=f"qraw{t}")
                            for ic in range(NIC):
                                ps = psq.tile([128, 512], F32)
                                for k in range(KT):
                                    nc.tensor.matmul(
                                        ps[:],
                                        mmcast(wq_sb[:, k,
                                                     t * 128:(t + 1) * 128]),
                                        mmcast(xT_sb[:, k,
                                                     ic * 512:(ic + 1) * 512]),
                                        start=(k == 0), stop=(k == KT - 1))
                                nc.vector.tensor_scalar_add(
                                    qraw[:, ic * 512:(ic + 1) * 512], ps[:],
                                    bq_sb[:, t:t + 1])
                            return qraw

                        # software pipeline: tile t's norm chain runs behind
                        # the raw matmuls of tile t+1 so the (in-order) PE
                        # queue never waits on the DVE/ACT chain.
                        qraws = {}
                        qsqs = {}
                        for t in range(TCH):
                            if t >= 1:
                                qsqs[t - 1] = q_sq(t - 1, qraws[t - 1])
                            qraws[t] = q_raw(t)
                            if t >= 1:
                                q_fin(t - 1, qraws[t - 1], qsqs[t - 1])

                    # ------- K projection + l2norm, V proj interleaved -----
                    # v matmuls are emitted between k tiles so the PE stays
                    # busy while the k-norm DVE/ACT chain drains.  q tile 3's
                    # chain is emitted behind the first k matmuls.
                    with tc.tile_pool(name="ktmp", bufs=1) as ktmp, \
                         tc.tile_pool(name="psv", bufs=2, space="PSUM") as psv, \
                         tc.tile_pool(name="psks", bufs=2, space="PSUM") as psks:

                        def k_sq(t, kraw):
                            sqk = ktmp.tile([128, Ly], F32, tag="sqk",
                                            bufs=1, name=f"sqk{t}")
                            nc.vector.tensor_mul(sqk[:].bitcast(F32R),
                                                 kraw[:], kraw[:])
                            return sqk

                        def k_fin(t, kraw, sqk):
                            rsk = ktmp.tile([128, Ly], F32, tag="rsk",
                                            name=f"rsk{t}")
                            for jc in range(Ly // 512):
                                ssq = psks.tile([128, 512], F32)
                                nc.tensor.matmul(
                                    ssq[:], sel_r[:].bitcast(F32R),
                                    sqk[:, jc * 512:(jc + 1) * 512]
                                    .bitcast(F32R),
                                    start=True, stop=True)
                                nc.scalar.activation(
                                    rsk[:, jc * 512:(jc + 1) * 512], ssq[:],
                                    Ln)
                            nc.scalar.activation(rsk[:], rsk[:], Exp,
                                                 scale=-0.5)
                            nc.gpsimd.tensor_mul(mmcast(kn_sb[:, t, :]),
                                                 kraw[:], rsk[:])

                        def v_proj(jt):
                            ps = psv.tile([128, 512], F32, tag="vps")
                            for k in range(KT):
                                nc.tensor.matmul(
                                    ps[:],
                                    mmcast(yT_sb[:, k,
                                                 jt * 128:(jt + 1) * 128]),
                                    mmcast(wv_sb[:, k, :]),
                                    start=(k == 0), stop=(k == KT - 1))
                            vslot = v_sb[:, jt, :].rearrange(
                                "p (h e) -> p h e", e=65)
                            nc.vector.tensor_copy(
                                mmcast(vslot[:, :, 0:64]),
                                ps[:].rearrange("p (h e) -> p h e", e=64))

                        kraws = {}
                        ksqs = {}

                        def kv_iter(t):
                            if t == 0:
                                qsqs[TCH - 1] = q_sq(TCH - 1, qraws[TCH - 1])
                            else:
                                ksqs[t - 1] = k_sq(t - 1, kraws[t - 1])
                            kraws[t] = kraw_p.tile([128, Ly], F32,
                                                   tag="kraw", bufs=3,
                                                   name=f"kraw{t}")
                            for jc in range(Ly // 512):
                                ps = psk.tile([128, 512], F32, tag="kps")
                                for k in range(KT):
                                    nc.tensor.matmul(
                                        ps[:],
                                        mmcast(wk_sb[:, k,
                                                     t * 128:(t + 1) * 128]),
                                        mmcast(yT_sb[:, k,
                                                     jc * 512:(jc + 1) * 512]),
                                        start=(k == 0), stop=(k == KT - 1))
                                nc.vector.tensor_copy(
                                    kraws[t][:, jc * 512:(jc + 1) * 512],
                                    ps[:])
                                if t == 0 and jc == 0:
                                    # q tile 3's norm chain, covered by the
                                    # k matmuls just emitted
                                    q_fin(TCH - 1, qraws[TCH - 1],
                                          qsqs[TCH - 1])

                            v_proj(4 * t)
                            v_proj(4 * t + 1)
                            if t >= 1:
                                k_fin(t - 1, kraws[t - 1], ksqs[t - 1])
                            if t == TCH - 1:
                                ksqs[t] = k_sq(t, kraws[t])
                            v_proj(4 * t + 2)
                            v_proj(4 * t + 3)
                            if t == TCH - 1:
                                # last k tile's chain, covered by the v
                                # matmuls emitted just above
                                k_fin(t, kraws[t], ksqs[t])

                        for t in range(TCH):
                            kv_iter(t)

            # ---------------- attention + output projection ----------------
            # psf lives in the outer scope: its 2 banks never alias the
            # sweep pools, so the final out-projection starts without any
            # pool-close barrier
            with tc.tile_pool(name="pp_p", bufs=6) as pp_p, \
                 tc.tile_pool(name="nrm_p", bufs=2) as nrm_p, \
                 tc.tile_pool(name="ost_p", bufs=2) as ost_p, \
                 tc.tile_pool(name="psf", bufs=2, space="PSUM") as psf:

                # dummy broadcast: pays the gpsimd ucode swap (TT ->
                # PartitionBroadcast, ~7us) while the Pool is idle, instead
                # of on the first sweep's normalize chain
                warm = nrm_p.tile([64, 8], F32, tag="warm", bufs=1)
                nc.gpsimd.partition_broadcast(warm[:], sel_f[0:1, 0:8])

                def normalize_a(ic, hp, den):
                    rec = nrm_p.tile([1, 1024], F32, tag="rec", bufs=4,
                                     name=f"rec{ic}_{hp}")
                    nc.scalar.activation(rec[0:1, :], den[0:1, :], Ln)
                    nc.scalar.activation(rec[0:1, :], rec[0:1, :], Exp,
                                         scale=-1.0)
                    bcts = []
                    for p in range(2):
                        bct = nrm_p.tile([64, 512], F32, tag="bct", bufs=4,
                                         name=f"bct{ic}_{hp}_{p}")
                        nc.gpsimd.partition_broadcast(
                            bct[:], rec[0:1, p * 512:(p + 1) * 512])
                        bcts.append(bct)
                    return bcts

                def normalize_b(ic, hp, oas, bcts):
                    # emitted well after normalize_a so the DVE queue never
                    # blocks on the Pool broadcasts
                    for p in range(2):
                        nc.vector.tensor_mul(
                            mmcast(oT_sb[p * 64:p * 64 + 64, hp,
                                         ic * 512:(ic + 1) * 512]),
                            oas[p][0:64, :], bcts[p][:])

                def sweep(pso, pss, ic, hp, ebs, pending):
                    """One (i-chunk, head-pair) attention pass.  `pending` is
                    the previous sweep's normalize closure: emitted a few jt
                    into this sweep so its ACT/DVE/Pool work hides behind
                    this sweep's exp/mul stream instead of gating it."""
                    opsums = [pso.tile([65, 512], F32, tag="opsum",
                                       name=f"opsum{_p}")
                              for _p in range(2)]
                    # software pipeline: scores(jt) are emitted before
                    # exp/mul/PV of jt-1 so the two K=64 score matmuls
                    # keep adjacent scheduler priority (they execute
                    # concurrently on disjoint PE row groups)
                    s2s = [None] * NJT

                    def tail(jt):
                        p0 = pp_p.tile([128, 1024], AD, tag="p0",
                                       name=f"p0_{jt}")
                        nc.scalar.activation(p0[:], s2s[jt][:], Exp)
                        pt = pp_p.tile([128, 1024], AD, tag="pt",
                                       name=f"pt_{jt}")
                        if bf:
                            nc.vector.tensor_mul(pt[:], p0[:], ebs[jt][:])
                        else:
                            nc.vector.tensor_mul(
                                mmcast(pt[:].rearrange(
                                    "q (p i) -> q p i", p=2)),
                                p0[:].rearrange("q (p i) -> q p i", p=2),
                                ebs[jt][:].unsqueeze(1).broadcast_to(
                                    [128, 2, 512]))
                        for p in range(2):
                            h = 2 * hp + p
                            nc.tensor.matmul(
                                opsums[p][:],
                                mmcast(v_sb[:, jt, h * 65:h * 65 + 65]),
                                mmcast(pt[:, p * 512:(p + 1) * 512]),
                                start=(jt == 0), stop=(jt == NJT - 1))

                    # the previous sweep's Ln/Exp + broadcasts are emitted
                    # FIRST: the ACT engine has an idle bubble at the sweep
                    # boundary (waiting on the first score matmuls), which
                    # absorbs the reciprocal chain for free
                    bcts_prev = None
                    if pending is not None:
                        bcts_prev = pending[0]()
                    for jt in range(NJT):
                        s2s[jt] = pss.tile([128, 1024], F32, tag="s",
                                           name=f"s2_{jt}")
                        for p in range(2):
                            nc.tensor.matmul(
                                s2s[jt][:, p * 512:(p + 1) * 512],
                                mmcast(kn_sb[p * 64:p * 64 + 64, hp,
                                             jt * 128:(jt + 1) * 128]),
                                mmcast(qn_sb[p * 64:p * 64 + 64, hp,
                                             ic * 512:(ic + 1) * 512]),
                                start=True, stop=True,
                                tile_position=(p * 64, 0) if bf else None)
                        if jt > 0:
                            tail(jt - 1)
                        if jt == 10 and pending is not None:
                            pending[1](bcts_prev)
                            pending = None
                    tail(NJT - 1)
                    if pending is not None:
                        pending[1](pending[0]())
                    oas = []
                    for p in range(2):
                        # copies release the PSUM accumulators quickly
                        oa = nrm_p.tile([65, 512], F32, tag="oa", bufs=4,
                                        name=f"oa{ic}_{hp}_{p}")
                        nc.vector.tensor_copy(oa[:], opsums[p][:])
                        oas.append(oa)
                    # denominators straight from PSUM row 64 (partition 64
                    # is an aligned start) so normalize_a never waits on oa
                    den = nrm_p.tile([1, 1024], F32, tag="den", bufs=4,
                                     name=f"den{ic}_{hp}")
                    for p in range(2):
                        nc.vector.tensor_copy(den[0:1, p * 512:(p + 1) * 512],
                                              opsums[p][64:65, :])
                    return (lambda: normalize_a(ic, hp, den),
                            lambda b: normalize_b(ic, hp, oas, b))

                def op_mms(ic, ct, ps, ks):
                    for k in ks:
                        nc.tensor.matmul(
                            ps[:],
                            mmcast(wo_sb[:, k, ct * 128:(ct + 1) * 128]),
                            mmcast(oT_sb[:, k, ic * 512:(ic + 1) * 512]),
                            start=(k == 0), stop=(k == TCH - 1))

                def op_drain(ic, ct, ps):
                    st = ost_p.tile([128, 512], F32, tag="ost")
                    nc.vector.tensor_copy(st[:], ps[:])
                    # alternate DGE queues so the final writeback drains
                    # twice as fast
                    eng = nc.sync if ct % 2 == 0 else nc.scalar
                    eng.dma_start(
                        out_d[:, ct, ic * 512:(ic + 1) * 512], st[:])

                def out_proj(ic, heads=None):
                    """heads: optional list of (ct, ps) with k=0..2 already
                    accumulated (emitted before the last normalize chain so
                    the PE chews on them while it drains)."""
                    for ct in range(C // 128):
                        if heads is not None and ct < len(heads):
                            ps = heads[ct][1]
                            op_mms(ic, ct, ps, [TCH - 1])
                        else:
                            ps = psf.tile([128, 512], F32, tag="fout")
                            op_mms(ic, ct, ps, range(TCH))
                        op_drain(ic, ct, ps)

                # sweep order: ic1's first sweep runs before out_proj(ic0)
                # so the PE never waits on ic0's trailing normalize chain.
                # pss/pso close before the final out-projection (psf never
                # aliases them, so no barrier).
                with tc.tile_pool(name="pss", bufs=2, space="PSUM") as pss, \
                     tc.tile_pool(name="pso", bufs=2, space="PSUM") as pso:
                    pending = None
                    for hp in range(TCH):
                        pending = sweep(pso, pss, 0, hp, ebs0, pending)
                    ebs1 = emit_eb(1)
                    pending = sweep(pso, pss, 1, 0, ebs1, pending)
                    out_proj(0)
                    for hp in range(1, TCH):
                        pending = sweep(pso, pss, 1, hp, ebs1, pending)
                    # partial k=0..2 accumulations for the first two ct
                    # tiles go out BEFORE the last normalize chain: ready
                    # PE work to overlap the ACT/Pool/DVE drain
                    heads = []
                    for ct in range(2):
                        ps = psf.tile([128, 512], F32, tag="fout")
                        op_mms(1, ct, ps, range(TCH - 1))
                        heads.append((ct, ps))
                    pending[1](pending[0]())
                out_proj(1, heads)

    nc.compile()
    return nc


def _get_compiled(attn_dt: str):
    if attn_dt not in _COMPILED:
        _COMPILED[attn_dt] = _build(attn_dt)
    return _COMPILED[attn_dt]


def kernel(x, y, attn_bias, Wq, bq, Wk, Wv, Wo, bo, scale_mul_log):
    global LAST_EXEC_NS
    attn_dt = ATTN_DT
    x = np.asarray(x, dtype=np.float32)
    y = np.asarray(y, dtype=np.float32)
    attn_bias = np.asarray(attn_bias, dtype=np.float32)
    Wq = np.asarray(Wq, dtype=np.float32)
    bq = np.asarray(bq, dtype=np.float32)
    Wk = np.asarray(Wk, dtype=np.float32)
    Wv = np.asarray(Wv, dtype=np.float32)
    Wo = np.asarray(Wo, dtype=np.float32)
    bo = np.asarray(bo, dtype=np.float32)
    scale_mul_log = np.asarray(scale_mul_log, dtype=np.float32)

    nc = _get_compiled(attn_dt)

    scale = np.exp(np.minimum(scale_mul_log.reshape(H_TOT), MAX_SCALE_MUL))
    dt_in = np.float32 if attn_dt == "f32" else ml_dtypes.bfloat16
    ebT = np.exp(attn_bias.T)
    ebT = np.ascontiguousarray(ebT.astype(dt_in))

    xTs = [x[b].T.astype(dt_in, order="C") for b in range(B)]
    yTs = [y[b].T.astype(dt_in, order="C") for b in range(B)]

    in_maps = []
    for c in range(N_CORES):
        b, g = c // 2, c % 2
        sl = slice(g * CHL, (g + 1) * CHL)
        s_loc = scale[g * HL:(g + 1) * HL]       # 8 local heads
        inv2 = 1.0 / (s_loc * s_loc)
        # invs2[p, t] = 1/s^2 of head (2t + p//64)
        invs2 = np.empty((128, TCH), dtype=np.float32)
        for t in range(TCH):
            invs2[0:64, t] = inv2[2 * t]
            invs2[64:128, t] = inv2[2 * t + 1]
        bq4 = np.ascontiguousarray(bq[sl].reshape(TCH, 128).T)
        in_maps.append({
            "xT": xTs[b],
            "yT": yTs[b],
            "wq": Wq[:, sl].astype(dt_in, order="C"),
            "wk": Wk[:, sl].astype(dt_in, order="C"),
            "wv": Wv[:, sl].astype(dt_in, order="C"),
            "wo": Wo[sl, :].astype(dt_in, order="C"),
            "bq4": bq4,
            "invs2": invs2,
            "ebT": ebT,
        })

    trace = os.environ.get("KERNEL_TRACE", "0") == "1"
    if trace:
        _ensure_ntff_hook()
    res = run_bass_kernel_spmd(nc, in_maps, core_ids=list(range(N_CORES)),
                               trace=trace)
    LAST_EXEC_NS = res.exec_time_ns
    global LAST_RES
    LAST_RES = res

    out = np.empty((B, Lq, C), dtype=np.float32)
    for b in range(B):
        out[b] = res.results[2 * b]["outT"].T
        out[b] += res.results[2 * b + 1]["outT"].T
    out += bo
    return out



# revision 79
# speedup vs baseline: 1.0096x; 1.0096x over previous
"""Cross-attention (cosine/l2-normalized, biased softmax) on 8 TRN2 NeuronCores.

Sharding: core c handles batch b = c//2 and head group g = c%2 (8 of 16 heads,
i.e. a 512-wide slice of the QKV projections / Wo rows).  Each core computes a
partial output (its heads' contribution through Wo); the host sums the two
partials per batch and adds bo.

All tensors are kept transposed on chip (channels on partitions):
  qT/kT = (Wx)^T computed as lhsT=W, rhs=x^T; v in natural [j, ch] layout via
  lhsT=y^T.  Scores are computed transposed S^T[j, i] (lhsT = kn^T slice,
  rhs = qn^T slice, K = 64), softmax runs unnormalized as exp(S)*exp(bias)
  with the denominator obtained from an extra all-ones column appended to V,
  and the division happens after the PV matmul (partition_broadcast + mul).
L2-norm denominators use a block-diagonal selector matmul (K=128) for the
per-head sum of squares and exp(-0.5*ln(x)) on the scalar engine (the DVE
reciprocal op is ~5x an ACT pass; ACT Rsqrt is disallowed).

With KERNEL_ATTN_DT=bf16 (default) the inputs/weights are host-cast to bf16
so every matmul runs at the PE's full bf16 rate and input DMA traffic is
halved; PSUM accumulation, l2-norm chains and softmax denominators stay f32.

Scheduling notes (engines execute in queue order with shallow OOO windows,
and the PE clock drops to 1.2 GHz after any idle gap, reaching 2.4 GHz only
after 3us of continuous work — so the emission order is arranged to keep
every queue's head ready):
  - DMA is issued per-k-tile, first-needed first; eb/wo prefetch during the
    projection phases.
  - Each l2-norm chain (DVE square -> selector matmul -> ACT ln/exp -> mul)
    is split and emitted behind the next tile's raw matmuls.
  - The attention phase is ACT(exp)-bound: per-sweep softmax normalization
    is p-batched and deferred into the next sweep (bct/oT-mul emitted at
    jt=3/10) so it never gates the exp stream; gpsimd runs ONLY
    partition_broadcasts there (a dummy broadcast prewarms the Q7 ucode --
    switching gpsimd op types mid-phase costs ~7us per switch).
  - ic1's first sweep is emitted before ic0's output projection so the PE
    never waits on the trailing normalize chain; psf has its own psum banks
    so the final out-projection starts barrier-free.
"""

import os
import numpy as np
import ml_dtypes

import concourse.bass as bass
import concourse.tile as tile
from concourse import bacc, mybir
from concourse.bass_utils import run_bass_kernel_spmd


def _ensure_ntff_hook():
    """Some containers ship an `antenv` stub without `axon_hooks`; trn_boot
    then skips installing the NTFF profile hook and run_bass_kernel_spmd
    crashes on `trace=True`.  Recreate the module + hook (trn_boot step 6)
    when — and only when — it is missing."""
    try:
        import antenv.axon_hooks  # noqa: F401
        return
    except ImportError:
        pass
    try:
        import sys
        import types
        import antenv
        from trn_agent_boot.trn_boot import _ntff_profile_via_ctypes
        mod = types.ModuleType("antenv.axon_hooks")
        _state = {"hook": None}
        mod.set_axon_ntff_profile_hook = lambda h: _state.__setitem__("hook", h)
        mod.get_axon_ntff_profile_hook = lambda: _state["hook"]
        sys.modules["antenv.axon_hooks"] = mod
        antenv.axon_hooks = mod
        hook = _ntff_profile_via_ctypes("/opt/axon/libaxon_pjrt.so")
        if hook is not None:
            mod.set_axon_ntff_profile_hook(hook)
    except Exception:
        pass

F32 = mybir.dt.float32
F32R = mybir.dt.float32r
BF16 = mybir.dt.bfloat16

B, Lq, Ly, C = 4, 1024, 2048, 1024
H_TOT, D = 16, 64
HL = 8           # heads per core
CHL = HL * D     # 512 channels per core
TCH = CHL // 128  # 4 channel tiles (2 heads each)
KT = C // 128     # 8 contraction tiles for the projections
NJT = Ly // 128   # 16 j tiles
NIC = Lq // 512   # 2 i chunks
N_CORES = 8
MAX_SCALE_MUL = float(np.log(100.0))

# attention dtype: "f32" (f32r matmuls, fp32 probs) or "bf16"
ATTN_DT = os.environ.get("KERNEL_ATTN_DT", "bf16")

LAST_EXEC_NS = None
LAST_RES = None
_COMPILED = {}
Exp = mybir.ActivationFunctionType.Exp
Ln = mybir.ActivationFunctionType.Ln

_ACT_TABLES_INSTALLED = False


def _install_act_tables():
    """Point both bacc and walrus at an act_info.json with the combined
    ln+exp function set first, so Ln/Exp alternation (softmax denominators,
    l2-norm rsqrt via exp(-0.5 ln x)) stops thrashing the ACT spline table
    (~1.3 us per reload).  Selection is first-match over the set list."""
    global _ACT_TABLES_INSTALLED
    if _ACT_TABLES_INSTALLED:
        return
    import json
    import shutil
    import tempfile
    import concourse.hw_specs as hw_specs
    import concourse.bacc as bacc_mod
    try:
        from neuronxcc.driver.Job import Job
        from neuronxcc.driver.jobs.support.FindActInfo import findActInfoFile
        src = findActInfoFile(Job.getPackageDir(), "gen3")
    except Exception:
        return
    dst = os.path.join(tempfile.mkdtemp(prefix="actpwp"), "pwp")
    shutil.copytree(os.path.dirname(src), dst)
    info_path = os.path.join(dst, "act_info.json")
    with open(info_path) as f:
        info = json.load(f)
    key = "natural_log_exp_and_others"
    info["act_func_sets"].sort(key=lambda s: 0 if s["name"] == key else 1)
    with open(info_path, "w") as f:
        json.dump(info, f)
    os.environ["BASS_ACT_ROOT_JSON_PATH"] = info_path

    orig = hw_specs.get_activation_tables

    def reordered(arch):
        d = orig(arch)
        if key not in d:
            return d
        out = {key: d[key]}
        out.update((k, v) for k, v in d.items() if k != key)
        return out

    hw_specs.get_activation_tables = reordered
    bacc_mod.get_activation_tables = reordered
    _ACT_TABLES_INSTALLED = True


def _build(attn_dt: str):
    _install_act_tables()
    bf = attn_dt != "f32"
    AD = BF16 if bf else F32

    def mmcast(ap):
        # matmul operand dtype for the attention matmuls
        return ap if bf else ap.bitcast(F32R)

    nc = bacc.Bacc("TRN2", target_bir_lowering=False, debug=False,
                   num_devices=N_CORES)

    xT_ap = nc.dram_tensor("xT", [C, Lq], AD, kind="ExternalInput").ap()
    yT_ap = nc.dram_tensor("yT", [C, Ly], AD, kind="ExternalInput").ap()
    wq_ap = nc.dram_tensor("wq", [C, CHL], AD, kind="ExternalInput").ap()
    wk_ap = nc.dram_tensor("wk", [C, CHL], AD, kind="ExternalInput").ap()
    wv_ap = nc.dram_tensor("wv", [C, CHL], AD, kind="ExternalInput").ap()
    wo_ap = nc.dram_tensor("wo", [CHL, C], AD, kind="ExternalInput").ap()
    bq_ap = nc.dram_tensor("bq4", [128, TCH], F32, kind="ExternalInput").ap()
    is2_ap = nc.dram_tensor("invs2", [128, TCH], F32, kind="ExternalInput").ap()
    eb_ap = nc.dram_tensor("ebT", [Ly, Lq], AD, kind="ExternalInput").ap()
    out_ap = nc.dram_tensor("outT", [C, Lq], F32, kind="ExternalOutput").ap()

    xT_d = xT_ap.rearrange("(k p) i -> p k i", p=128)
    yT_d = yT_ap.rearrange("(k p) j -> p k j", p=128)
    wq_d = wq_ap.rearrange("(k p) n -> p k n", p=128)
    wk_d = wk_ap.rearrange("(k p) n -> p k n", p=128)
    wv_d = wv_ap.rearrange("(k p) n -> p k n", p=128)
    wo_d = wo_ap.rearrange("(k p) n -> p k n", p=128)
    eb_d = eb_ap.rearrange("(jt p) i -> p jt i", p=128)
    out_d = out_ap.rearrange("(ct p) i -> p ct i", p=128)

    with tile.TileContext(nc) as tc:
        with tc.tile_pool(name="persist", bufs=1) as pers, \
             tc.tile_pool(name="qn_p", bufs=1) as qn_p, \
             tc.tile_pool(name="kn_p", bufs=1) as kn_p, \
             tc.tile_pool(name="v_p", bufs=1) as v_p, \
             tc.tile_pool(name="wo_p", bufs=1) as wo_p, \
             tc.tile_pool(name="oT_p", bufs=1) as oT_p, \
             tc.tile_pool(name="eb_p", bufs=17) as eb_p:

            # block-diagonal parity selector: sel.T @ sq sums each 64-row
            # head block and replicates the sums over that block's rows
            sel_f = pers.tile([128, 128], F32)
            nc.gpsimd.memset(sel_f[:], 0.0)
            nc.gpsimd.memset(sel_f[0:64, 0:64], 1.0)
            nc.gpsimd.memset(sel_f[64:128, 64:128], 1.0)
            sel_r = pers.tile([128, 128], F32)
            nc.vector.tensor_copy(sel_r[:].bitcast(F32R), sel_f[:])
            bq_sb = pers.tile([128, TCH], F32)
            nc.sync.dma_start(bq_sb[:], bq_ap[:])
            is2_sb = pers.tile([128, TCH], F32)
            nc.sync.dma_start(is2_sb[:], is2_ap[:])

            qn_sb = qn_p.tile([128, TCH, Lq], AD)     # qn^T
            kn_sb = kn_p.tile([128, TCH, Ly], AD)     # kn^T
            v_sb = v_p.tile([128, NJT, HL * 65], AD)  # v (+ ones col per head)
            # the per-head ones columns never change: write them once
            nc.gpsimd.memset(
                v_sb[:].rearrange("p j (h e) -> p j h e", e=65)[:, :, :, 64:65],
                1.0)

            # attention-phase tensors are allocated up front so the eb / wo
            # prefetch can overlap the projection phases
            wo_sb = wo_p.tile([128, TCH, C], AD)
            oT_sb = oT_p.tile([128, TCH, Lq], AD)

            def emit_eb(ic):
                ebs = []
                for jt in range(NJT):
                    if bf:
                        # duplicated halves so the prob multiply is a
                        # plain step-1 2D op (DVE 2x bf16 mode)
                        ebt = eb_p.tile([128, 1024], AD, tag="eb",
                                        name=f"eb{ic}_{jt}")
                        nc.sync.dma_start(
                            ebt[:, 0:512],
                            eb_d[:, jt, ic * 512:(ic + 1) * 512])
                        nc.sync.dma_start(
                            ebt[:, 512:1024],
                            eb_d[:, jt, ic * 512:(ic + 1) * 512])
                    else:
                        ebt = eb_p.tile([128, 512], AD, tag="eb",
                                        name=f"eb{ic}_{jt}")
                        nc.sync.dma_start(
                            ebt[:], eb_d[:, jt, ic * 512:(ic + 1) * 512])
                    ebs.append(ebt)
                return ebs

            # yT/wv/wk space is reserved up front; their DMAs are emitted
            # after the q-phase loads so x/wq win the DMA queues first.
            with tc.tile_pool(name="yT_p", bufs=1) as yT_p, \
                 tc.tile_pool(name="wv_p", bufs=1) as wv_p, \
                 tc.tile_pool(name="wk_p", bufs=1) as wk_p:
                yT_sb = yT_p.tile([128, KT, Ly], AD)
                wv_sb = wv_p.tile([128, KT, CHL], AD)
                wk_sb = wk_p.tile([128, KT, CHL], AD)

                # -------------- Q projection + l2norm(+scale) --------------
                # qtmp/qnrm/psqs outlive the Q block: q tile 3's norm chain
                # is emitted inside the K phase (PSUM: Q 4+2, KV 2+2+2+2).
                # kraw/psk are allocated below the Q-phase pools so the
                # first K matmuls never wait on a space-reuse barrier
                # against xT/wq/psq (PSUM: Q 4+2+2 = 8, KV 2+2+2+2 = 8)
                with tc.tile_pool(name="qtmp", bufs=2) as qtmp, \
                     tc.tile_pool(name="qnrm", bufs=1) as qnrm, \
                     tc.tile_pool(name="kraw_p", bufs=2) as kraw_p, \
                     tc.tile_pool(name="psqs", bufs=2, space="PSUM") as psqs, \
                     tc.tile_pool(name="psk", bufs=2, space="PSUM") as psk:

                    def q_sq(t, qraw):
                        sq = qnrm.tile([128, Lq], F32, tag="sq",
                                       name=f"sq{t}")
                        nc.vector.tensor_mul(sq[:].bitcast(F32R), qraw[:],
                                             qraw[:])
                        return sq

                    def q_fin(t, qraw, sq):
                        rs = qnrm.tile([128, Lq], F32, tag="rs",
                                       name=f"rs{t}")
                        for ic in range(NIC):
                            ssq = psqs.tile([128, 512], F32)
                            nc.tensor.matmul(
                                ssq[:], sel_r[:].bitcast(F32R),
                                sq[:, ic * 512:(ic + 1) * 512].bitcast(F32R),
                                start=True, stop=True)
                            # s_h/|q| = exp(-0.5*ln(sumsq/s_h^2))
                            nc.scalar.activation(
                                rs[:, ic * 512:(ic + 1) * 512], ssq[:], Ln,
                                scale=is2_sb[:, t:t + 1])
                        nc.scalar.activation(rs[:], rs[:], Exp, scale=-0.5)
                        # Pool: keeps the in-order DVE queue free for the
                        # psum-draining adds/copies (Pool only runs
                        # tensor_tensor during the projection phases, so no
                        # gpsimd ucode swap cost)
                        nc.gpsimd.tensor_mul(mmcast(qn_sb[:, t, :]),
                                             qraw[:], rs[:])

                    with tc.tile_pool(name="xT_p", bufs=1) as xT_p, \
                         tc.tile_pool(name="wq_p", bufs=1) as wq_p, \
                         tc.tile_pool(name="psq", bufs=4, space="PSUM") as psq:
                        wq_sb = wq_p.tile([128, KT, CHL], AD)
                        xT_sb = xT_p.tile([128, KT, Lq], AD)
                        # per-k interleave so the first Q matmul's operands
                        # land as early as possible
                        for k in range(KT):
                            nc.sync.dma_start(wq_sb[:, k, :], wq_d[:, k, :])
                            nc.sync.dma_start(xT_sb[:, k, :], xT_d[:, k, :])
                        for k in range(KT):
                            nc.sync.dma_start(wk_sb[:, k, :], wk_d[:, k, :])
                            nc.sync.dma_start(yT_sb[:, k, :], yT_d[:, k, :])
                        nc.sync.dma_start(wv_sb[:], wv_d[:])
                        # prefetch ic=0 bias tiles and wo during projections
                        ebs0 = emit_eb(0)
                        nc.sync.dma_start(wo_sb[:], wo_d[:])

                        def q_raw(t):
                            qraw = qtmp.tile([128, Lq], F32, tag="qraw",
                                             name=f"qraw{t}")
                            for ic in range(NIC):
                                ps = psq.tile([128, 512], F32)
                                for k in range(KT):
                                    nc.tensor.matmul(
                                        ps[:],
                                        mmcast(wq_sb[:, k,
                                                     t * 128:(t + 1) * 128]),
                                        mmcast(xT_sb[:, k,
                                                     ic * 512:(ic + 1) * 512]),
                                        start=(k == 0), stop=(k == KT - 1))
                                nc.vector.tensor_scalar_add(
                                    qraw[:, ic * 512:(ic + 1) * 512], ps[:],
                                    bq_sb[:, t:t + 1])
                            return qraw

                        # software pipeline: tile t's norm chain runs behind
                        # the raw matmuls of tile t+1 so the (in-order) PE
                        # queue never waits on the DVE/ACT chain.
                        qraws = {}
                        qsqs = {}
                        for t in range(TCH):
                            if t >= 1:
                                qsqs[t - 1] = q_sq(t - 1, qraws[t - 1])
                            qraws[t] = q_raw(t)
                            if t >= 1:
                                q_fin(t - 1, qraws[t - 1], qsqs[t - 1])

                    # ------- K projection + l2norm, V proj interleaved -----
                    # v matmuls are emitted between k tiles so the PE stays
                    # busy while the k-norm DVE/ACT chain drains.  q tile 3's
                    # chain is emitted behind the first k matmuls.
                    with tc.tile_pool(name="ktmp", bufs=1) as ktmp, \
                         tc.tile_pool(name="psv", bufs=2, space="PSUM") as psv, \
                         tc.tile_pool(name="psks", bufs=2, space="PSUM") as psks:

                        def k_sq(t, kraw):
                            sqk = ktmp.tile([128, Ly], F32, tag="sqk",
                                            bufs=1, name=f"sqk{t}")
                            nc.vector.tensor_mul(sqk[:].bitcast(F32R),
                                                 kraw[:], kraw[:])
                            return sqk

                        def k_fin(t, kraw, sqk):
                            rsk = ktmp.tile([128, Ly], F32, tag="rsk",
                                            name=f"rsk{t}")
                            for jc in range(Ly // 512):
                                ssq = psks.tile([128, 512], F32)
                                nc.tensor.matmul(
                                    ssq[:], sel_r[:].bitcast(F32R),
                                    sqk[:, jc * 512:(jc + 1) * 512]
                                    .bitcast(F32R),
                                    start=True, stop=True)
                                nc.scalar.activation(
                                    rsk[:, jc * 512:(jc + 1) * 512], ssq[:],
                                    Ln)
                            nc.scalar.activation(rsk[:], rsk[:], Exp,
                                                 scale=-0.5)
                            nc.gpsimd.tensor_mul(mmcast(kn_sb[:, t, :]),
                                                 kraw[:], rsk[:])

                        def v_proj(jt):
                            ps = psv.tile([128, 512], F32, tag="vps")
                            for k in range(KT):
                                nc.tensor.matmul(
                                    ps[:],
                                    mmcast(yT_sb[:, k,
                                                 jt * 128:(jt + 1) * 128]),
                                    mmcast(wv_sb[:, k, :]),
                                    start=(k == 0), stop=(k == KT - 1))
                            vslot = v_sb[:, jt, :].rearrange(
                                "p (h e) -> p h e", e=65)
                            nc.vector.tensor_copy(
                                mmcast(vslot[:, :, 0:64]),
                                ps[:].rearrange("p (h e) -> p h e", e=64))

                        kraws = {}
                        ksqs = {}

                        def kv_iter(t):
                            if t == 0:
                                qsqs[TCH - 1] = q_sq(TCH - 1, qraws[TCH - 1])
                            else:
                                ksqs[t - 1] = k_sq(t - 1, kraws[t - 1])
                            kraws[t] = kraw_p.tile([128, Ly], F32,
                                                   tag="kraw", bufs=3,
                                                   name=f"kraw{t}")
                            for jc in range(Ly // 512):
                                ps = psk.tile([128, 512], F32, tag="kps")
                                for k in range(KT):
                                    nc.tensor.matmul(
                                        ps[:],
                                        mmcast(wk_sb[:, k,
                                                     t * 128:(t + 1) * 128]),
                                        mmcast(yT_sb[:, k,
                                                     jc * 512:(jc + 1) * 512]),
                                        start=(k == 0), stop=(k == KT - 1))
                                nc.vector.tensor_copy(
                                    kraws[t][:, jc * 512:(jc + 1) * 512],
                                    ps[:])
                                if t == 0 and jc == 0:
                                    # q tile 3's norm chain, covered by the
                                    # k matmuls just emitted
                                    q_fin(TCH - 1, qraws[TCH - 1],
                                          qsqs[TCH - 1])

                            v_proj(4 * t)
                            v_proj(4 * t + 1)
                            if t >= 1:
                                k_fin(t - 1, kraws[t - 1], ksqs[t - 1])
                            if t == TCH - 1:
                                ksqs[t] = k_sq(t, kraws[t])
                            v_proj(4 * t + 2)
                            v_proj(4 * t + 3)
                            if t == TCH - 1:
                                # last k tile's chain, covered by the v
                                # matmuls emitted just above
                                k_fin(t, kraws[t], ksqs[t])

                        for t in range(TCH):
                            kv_iter(t)

            # ---------------- attention + output projection ----------------
            # psf lives in the outer scope: its 2 banks never alias the
            # sweep pools, so the final out-projection starts without any
            # pool-close barrier
            with tc.tile_pool(name="pp_p", bufs=6) as pp_p, \
                 tc.tile_pool(name="nrm_p", bufs=2) as nrm_p, \
                 tc.tile_pool(name="ost_p", bufs=2) as ost_p, \
                 tc.tile_pool(name="psf", bufs=2, space="PSUM") as psf:

                # dummy broadcast: pays the gpsimd ucode swap (TT ->
                # PartitionBroadcast, ~7us) while the Pool is idle, instead
                # of on the first sweep's normalize chain
                warm = nrm_p.tile([64, 8], F32, tag="warm", bufs=1)
                nc.gpsimd.partition_broadcast(warm[:], sel_f[0:1, 0:8])

                def normalize_a(ic, hp, den):
                    rec = nrm_p.tile([1, 1024], F32, tag="rec", bufs=4,
                                     name=f"rec{ic}_{hp}")
                    nc.scalar.activation(rec[0:1, :], den[0:1, :], Ln)
                    nc.scalar.activation(rec[0:1, :], rec[0:1, :], Exp,
                                         scale=-1.0)
                    bcts = []
                    for p in range(2):
                        bct = nrm_p.tile([64, 512], F32, tag="bct", bufs=4,
                                         name=f"bct{ic}_{hp}_{p}")
                        nc.gpsimd.partition_broadcast(
                            bct[:], rec[0:1, p * 512:(p + 1) * 512])
                        bcts.append(bct)
                    return bcts

                def normalize_b(ic, hp, oas, bcts):
                    # emitted well after normalize_a so the DVE queue never
                    # blocks on the Pool broadcasts
                    for p in range(2):
                        nc.vector.tensor_mul(
                            mmcast(oT_sb[p * 64:p * 64 + 64, hp,
                                         ic * 512:(ic + 1) * 512]),
                            oas[p][0:64, :], bcts[p][:])

                def sweep(pso, pss, ic, hp, ebs, pending):
                    """One (i-chunk, head-pair) attention pass.  `pending` is
                    the previous sweep's normalize closure: emitted a few jt
                    into this sweep so its ACT/DVE/Pool work hides behind
                    this sweep's exp/mul stream instead of gating it."""
                    opsums = [pso.tile([65, 512], F32, tag="opsum",
                                       name=f"opsum{_p}")
                              for _p in range(2)]
                    # software pipeline: scores(jt) are emitted before
                    # exp/mul/PV of jt-1 so the two K=64 score matmuls
                    # keep adjacent scheduler priority (they execute
                    # concurrently on disjoint PE row groups)
                    s2s = [None] * NJT

                    def tail(jt):
                        p0 = pp_p.tile([128, 1024], AD, tag="p0",
                                       name=f"p0_{jt}")
                        nc.scalar.activation(p0[:], s2s[jt][:], Exp)
                        pt = pp_p.tile([128, 1024], AD, tag="pt",
                                       name=f"pt_{jt}")
                        if bf:
                            nc.vector.tensor_mul(pt[:], p0[:], ebs[jt][:])
                        else:
                            nc.vector.tensor_mul(
                                mmcast(pt[:].rearrange(
                                    "q (p i) -> q p i", p=2)),
                                p0[:].rearrange("q (p i) -> q p i", p=2),
                                ebs[jt][:].unsqueeze(1).broadcast_to(
                                    [128, 2, 512]))
                        for p in range(2):
                            h = 2 * hp + p
                            nc.tensor.matmul(
                                opsums[p][:],
                                mmcast(v_sb[:, jt, h * 65:h * 65 + 65]),
                                mmcast(pt[:, p * 512:(p + 1) * 512]),
                                start=(jt == 0), stop=(jt == NJT - 1))

                    # the previous sweep's Ln/Exp + broadcasts are emitted
                    # FIRST: the ACT engine has an idle bubble at the sweep
                    # boundary (waiting on the first score matmuls), which
                    # absorbs the reciprocal chain for free
                    bcts_prev = None
                    if pending is not None:
                        bcts_prev = pending[0]()
                    for jt in range(NJT):
                        s2s[jt] = pss.tile([128, 1024], F32, tag="s",
                                           name=f"s2_{jt}")
                        for p in range(2):
                            nc.tensor.matmul(
                                s2s[jt][:, p * 512:(p + 1) * 512],
                                mmcast(kn_sb[p * 64:p * 64 + 64, hp,
                                             jt * 128:(jt + 1) * 128]),
                                mmcast(qn_sb[p * 64:p * 64 + 64, hp,
                                             ic * 512:(ic + 1) * 512]),
                                start=True, stop=True,
                                tile_position=(p * 64, 0) if bf else None)
                        if jt > 0:
                            tail(jt - 1)
                        if jt == 10 and pending is not None:
                            pending[1](bcts_prev)
                            pending = None
                    tail(NJT - 1)
                    if pending is not None:
                        pending[1](pending[0]())
                    oas = []
                    for p in range(2):
                        # copies release the PSUM accumulators quickly
                        oa = nrm_p.tile([65, 512], F32, tag="oa", bufs=4,
                                        name=f"oa{ic}_{hp}_{p}")
                        nc.vector.tensor_copy(oa[:], opsums[p][:])
                        oas.append(oa)
                    # denominators straight from PSUM row 64 (partition 64
                    # is an aligned start) so normalize_a never waits on oa
                    den = nrm_p.tile([1, 1024], F32, tag="den", bufs=4,
                                     name=f"den{ic}_{hp}")
                    for p in range(2):
                        nc.vector.tensor_copy(den[0:1, p * 512:(p + 1) * 512],
                                              opsums[p][64:65, :])
                    return (lambda: normalize_a(ic, hp, den),
                            lambda b: normalize_b(ic, hp, oas, b))

                def op_mms(ic, ct, ps, ks):
                    for k in ks:
                        nc.tensor.matmul(
                            ps[:],
                            mmcast(wo_sb[:, k, ct * 128:(ct + 1) * 128]),
                            mmcast(oT_sb[:, k, ic * 512:(ic + 1) * 512]),
                            start=(k == 0), stop=(k == TCH - 1))

                def op_drain(ic, ct, ps):
                    st = ost_p.tile([128, 512], F32, tag="ost")
                    nc.vector.tensor_copy(st[:], ps[:])
                    # alternate DGE queues so the final writeback drains
                    # twice as fast
                    eng = nc.sync if ct % 2 == 0 else nc.scalar
                    eng.dma_start(
                        out_d[:, ct, ic * 512:(ic + 1) * 512], st[:])

                def out_proj(ic, heads=None):
                    """heads: optional list of (ct, ps) with k=0..2 already
                    accumulated (emitted before the last normalize chain so
                    the PE chews on them while it drains)."""
                    for ct in range(C // 128):
                        if heads is not None and ct < len(heads):
                            ps = heads[ct][1]
                            op_mms(ic, ct, ps, [TCH - 1])
                        else:
                            ps = psf.tile([128, 512], F32, tag="fout")
                            op_mms(ic, ct, ps, range(TCH))
                        op_drain(ic, ct, ps)

                # sweep order: ic1's first sweep runs before out_proj(ic0)
                # so the PE never waits on ic0's trailing normalize chain.
                # pss/pso close before the final out-projection (psf never
                # aliases them, so no barrier).
                with tc.tile_pool(name="pss", bufs=2, space="PSUM") as pss, \
                     tc.tile_pool(name="pso", bufs=2, space="PSUM") as pso:
                    pending = None
                    for hp in range(TCH):
                        pending = sweep(pso, pss, 0, hp, ebs0, pending)
                    ebs1 = emit_eb(1)
                    pending = sweep(pso, pss, 1, 0, ebs1, pending)
                    out_proj(0)
                    for hp in range(1, TCH):
                        pending = sweep(pso, pss, 1, hp, ebs1, pending)
                    pending[1](pending[0]())
                out_proj(1)

    nc.compile()
    return nc


def _get_compiled(attn_dt: str):
    if attn_dt not in _COMPILED:
        _COMPILED[attn_dt] = _build(attn_dt)
    return _COMPILED[attn_dt]


def kernel(x, y, attn_bias, Wq, bq, Wk, Wv, Wo, bo, scale_mul_log):
    global LAST_EXEC_NS
    attn_dt = ATTN_DT
    x = np.asarray(x, dtype=np.float32)
    y = np.asarray(y, dtype=np.float32)
    attn_bias = np.asarray(attn_bias, dtype=np.float32)
    Wq = np.asarray(Wq, dtype=np.float32)
    bq = np.asarray(bq, dtype=np.float32)
    Wk = np.asarray(Wk, dtype=np.float32)
    Wv = np.asarray(Wv, dtype=np.float32)
    Wo = np.asarray(Wo, dtype=np.float32)
    bo = np.asarray(bo, dtype=np.float32)
    scale_mul_log = np.asarray(scale_mul_log, dtype=np.float32)

    nc = _get_compiled(attn_dt)

    scale = np.exp(np.minimum(scale_mul_log.reshape(H_TOT), MAX_SCALE_MUL))
    dt_in = np.float32 if attn_dt == "f32" else ml_dtypes.bfloat16
    ebT = np.exp(attn_bias.T)
    ebT = np.ascontiguousarray(ebT.astype(dt_in))

    xTs = [x[b].T.astype(dt_in, order="C") for b in range(B)]
    yTs = [y[b].T.astype(dt_in, order="C") for b in range(B)]

    in_maps = []
    for c in range(N_CORES):
        b, g = c // 2, c % 2
        sl = slice(g * CHL, (g + 1) * CHL)
        s_loc = scale[g * HL:(g + 1) * HL]       # 8 local heads
        inv2 = 1.0 / (s_loc * s_loc)
        # invs2[p, t] = 1/s^2 of head (2t + p//64)
        invs2 = np.empty((128, TCH), dtype=np.float32)
        for t in range(TCH):
            invs2[0:64, t] = inv2[2 * t]
            invs2[64:128, t] = inv2[2 * t + 1]
        bq4 = np.ascontiguousarray(bq[sl].reshape(TCH, 128).T)
        in_maps.append({
            "xT": xTs[b],
            "yT": yTs[b],
            "wq": Wq[:, sl].astype(dt_in, order="C"),
            "wk": Wk[:, sl].astype(dt_in, order="C"),
            "wv": Wv[:, sl].astype(dt_in, order="C"),
            "wo": Wo[sl, :].astype(dt_in, order="C"),
            "bq4": bq4,
            "invs2": invs2,
            "ebT": ebT,
        })

    trace = os.environ.get("KERNEL_TRACE", "0") == "1"
    if trace:
        _ensure_ntff_hook()
    res = run_bass_kernel_spmd(nc, in_maps, core_ids=list(range(N_CORES)),
                               trace=trace)
    LAST_EXEC_NS = res.exec_time_ns
    global LAST_RES
    LAST_RES = res

    out = np.empty((B, Lq, C), dtype=np.float32)
    for b in range(B):
        out[b] = res.results[2 * b]["outT"].T
        out[b] += res.results[2 * b + 1]["outT"].T
    out += bo
    return out

